# revision 20
# baseline (speedup 1.0000x reference)
"""CGCN (GAT message-passing) Trainium2 kernel — 8-core SPMD.

Strategy
--------
Edges are sharded by destination node (host-side sort), so every segment
softmax is core-local: no all-reduce at all during the 3 routing passes.
Per core, destination nodes are degree-sorted into 128-row tiles with a
uniform slot count D per tile ("degree-packed" layout [128, T, D, 64]),
which turns gather/softmax/aggregate into a handful of large fused
vector-engine ops per strip with ~2% padding.

x_j rows are fetched once per strip via a two-level dma_gather (sorted
unique rows from the all-gathered table via <=4 windowed int16 gathers
into a compact DRAM strip table, then one slot-ordered gather), and all
4 forward passes run on the SBUF-resident strip.  Only two collectives:
AllGather of item features (after the on-device MLP) and AllGather of
final user preferences (before the reverse pass).

Softmax max-subtraction is dropped: every node vector is L2-normalized,
so |logit| <= 1 and exp() cannot overflow; results match the reference
to fp32 rounding.
"""
import os
import sys

sys.path.insert(0, "/opt/trn_rl_repo")

import numpy as np

import concourse.bass as bass
import concourse.bacc as bacc
import concourse.tile as tile
from concourse import mybir
from concourse.masks import make_identity

F32 = mybir.dt.float32
F32R = mybir.dt.float32r
I16 = mybir.dt.int16
ALU = mybir.AluOpType
ACTF = mybir.ActivationFunctionType
AXX = mybir.AxisListType.X

NCORES = 8
P = 128
C = 64               # node feature dim
NEG_SLOPE = 0.01
WINDOW = 32768       # int16 index window for dma_gather
SLOT_BUDGET = 80     # max T*D per strip  (SBUF sizing)
TMAX = 12            # max tiles per strip

_last_run_info = {}


# ----------------------------------------------------------------------------
# host-side structure building
# ----------------------------------------------------------------------------

def _wrap_idx(idx, n_pad):
    """dma_gather index layout: [128, n_pad/16] int16; element i at
    [i%16, i//16], replicated to partitions 16..127 (8 gpsimd cores)."""
    idx = np.asarray(idx, np.int64)
    assert n_pad % 16 == 0
    a = np.zeros((16, n_pad // 16), np.int16)
    if len(idx):
        pos = np.arange(len(idx))
        a[pos % 16, pos // 16] = idx.astype(np.int16)
    return np.tile(a, (8, 1))


def _build_side(dst, src_rows, eid, n_dst, table_rows):
    """Build the degree-packed slot structure for one direction.

    dst:       [E] destination node id (0 .. n_dst), segments owner-sharded
    src_rows:  [E] gather-table row per edge (into a table of `table_rows` rows)
    eid:       [E] edge id for alpha output mapping
    n_dst:     total destination nodes (multiple of NCORES)
    Returns a dict with the full per-core static structure + data arrays.
    """
    npc = n_dst // NCORES                      # nodes per core
    npc_pad = ((npc + P - 1) // P) * P
    T_tot = npc_pad // P

    deg = np.bincount(dst, minlength=n_dst)
    owner = dst // npc

    # per-core degree sort (stable desc)
    ords = []           # sorted-pos -> local node id
    invs = []           # local node id -> sorted-pos
    degs = np.zeros((NCORES, npc_pad), np.int64)
    for c in range(NCORES):
        d = deg[c * npc:(c + 1) * npc]
        o = np.argsort(-d, kind="stable")
        inv = np.empty(npc, np.int64)
        inv[o] = np.arange(npc)
        ords.append(o)
        invs.append(inv)
        degs[c, :npc] = d[o]

    # uniform tile D across cores
    D_t = degs.reshape(NCORES, T_tot, P).max(axis=2).max(axis=0)  # [T_tot]
    D_t = np.maximum(D_t, 1)

    # strips: consecutive tiles, T*max(D) <= SLOT_BUDGET, T <= TMAX
    strips = []         # (t0, T, D)
    t0 = 0
    while t0 < T_tot:
        Dmax = int(D_t[t0])
        T = 1
        while (t0 + T < T_tot and T < TMAX
               and (T + 1) * max(Dmax, int(D_t[t0 + T])) <= SLOT_BUDGET):
            Dmax = max(Dmax, int(D_t[t0 + T]))
            T += 1
        strips.append((t0, T, Dmax))
        t0 += T

    # per-edge CSR in (owner, sorted-pos) order
    spos = np.empty(len(dst), np.int64)
    for c in range(NCORES):
        m = owner == c
        spos[m] = invs[c][dst[m] - c * npc]
    order = np.lexsort((eid, spos, owner))
    e_src = src_rows[order]
    e_eid = eid[order]
    e_key = owner[order] * npc_pad + spos[order]
    starts = np.searchsorted(e_key, np.arange(NCORES * npc_pad) +
                             np.zeros(1, np.int64))  # [NCORES*npc_pad]

    nwin = (table_rows + WINDOW - 1) // WINDOW

    # per strip static schedule (shared by all cores): K_w, offsets
    side = {
        "npc": npc, "npc_pad": npc_pad, "T_tot": T_tot, "strips": strips,
        "ords": ords, "invs": invs, "deg": deg, "nwin": nwin,
        "l1_counts": [],   # per strip: [nwin] padded counts 128*K_w
        "slot_cols": [],   # per strip: S = T*D
    }

    # data arrays per core (built as lists of per-strip pieces, concatenated)
    l1_idx = [[] for _ in range(NCORES)]
    l2_idx = [[] for _ in range(NCORES)]
    masks = [[] for _ in range(NCORES)]
    slot_eids = [[] for _ in range(NCORES)]   # -1 for pads, else edge id

    for (t0, T, D) in strips:
        S = T * D
        side["slot_cols"].append(S)
        # gather per-core slot src rows [P, S] (p, t*D+j)
        slot_src_c = []
        for c in range(NCORES):
            k = np.arange(t0 * P, (t0 + T) * P)       # sorted positions
            st = starts[c * npc_pad + k]              # [T*P]
            dg = degs[c, k]
            j = np.arange(D)
            idxf = st[:, None] + j[None, :]           # [T*P, D]
            valid = j[None, :] < dg[:, None]
            idxf = np.where(valid, idxf, 0)
            ssrc = np.where(valid, e_src[idxf], -1)   # [T*P, D]
            seid = np.where(valid, e_eid[idxf], -1)
            # reshape to [P, T*D]: position (p, t*D+j) <- row t*P+p
            ssrc = ssrc.reshape(T, P, D).transpose(1, 0, 2).reshape(P, S)
            seid = seid.reshape(T, P, D).transpose(1, 0, 2).reshape(P, S)
            vmask = (ssrc >= 0)
            slot_src_c.append(ssrc)
            masks[c].append(vmask.astype(np.float32))
            slot_eids[c].append(seid)

        # level-1: per core unique sorted; pad per window to core-max K_w
        uniq_c = []
        win_counts = np.zeros((NCORES, nwin), np.int64)
        for c in range(NCORES):
            u = np.unique(slot_src_c[c][slot_src_c[c] >= 0])
            if len(u) == 0:
                u = np.array([0], np.int64)
            uniq_c.append(u)
            w = u // WINDOW
            win_counts[c] = np.bincount(w, minlength=nwin)
        K_w = ((win_counts.max(axis=0) + P - 1) // P).astype(np.int64)  # blocks
        side["l1_counts"].append([int(P * kw) for kw in K_w])
        K_tot = int(K_w.sum())
        assert K_tot * P < WINDOW, f"strip table too large: {K_tot * P}"

        for c in range(NCORES):
            u = uniq_c[c]
            w = u // WINDOW
            # positions of this core's uniques in the padded concatenated runs
            table_pos = np.full(len(u), 0, np.int64)
            l1_parts = []
            koff = 0
            off = 0
            for wi in range(nwin):
                n_w = int(win_counts[c, wi])
                run = u[off:off + n_w] - wi * WINDOW
                off += n_w
                n_pad = int(P * K_w[wi])
                if n_pad == 0:
                    continue
                runp = np.zeros(n_pad, np.int64)
                runp[:n_w] = run
                if n_w > 0:
                    runp[n_w:] = run[-1] if n_w else 0
                l1_parts.append(_wrap_idx(runp, n_pad))
                # table position of element q of this window run:
                q = np.arange(n_w)
                gpos = q  # within-window gather position
                # gathered row i -> partition i%128, block koff + i//128
                # dram row = p*K_tot + k
                table_pos[off - n_w:off] = (gpos % P) * K_tot + koff + gpos // P
                koff += int(K_w[wi])
            l1_idx[c].append(np.concatenate(l1_parts, axis=1)
                             if l1_parts else np.zeros((P, 0), np.int16))

            # level-2 index per slot: map src_row -> table row
            ssrc = slot_src_c[c]
            flat = ssrc.ravel()
            pos = np.searchsorted(u, np.where(flat >= 0, flat, u[0]))
            l2v = table_pos[pos]                       # [P*S]
            l2v = np.where(flat >= 0, l2v, 0)
            l2m = l2v.reshape(P, S)
            # gather position i = col*128 + p  (out [128, S, 64])
            n2 = P * S
            order2 = (np.arange(n2) % P) * S + (np.arange(n2) // P)
            l2_list = l2m.ravel()[order2]              # value at gather pos i
            l2_idx[c].append(_wrap_idx(l2_list, n2))

    side["l1_idx"] = [np.concatenate(x, axis=1) for x in l1_idx]
    side["l2_idx"] = [np.concatenate(x, axis=1) for x in l2_idx]
    side["masks"] = [np.concatenate(x, axis=1) for x in masks]
    side["slot_eids"] = [np.concatenate(x, axis=1) for x in slot_eids]
    return side


# ----------------------------------------------------------------------------
# device program
# ----------------------------------------------------------------------------

def _emit_strip_passes(nc, sb, pref_t, xj, mask_t, T, D, S, n_pass,
                      alpha_out_slices=None, tag=""):
    """Emit n_pass GAT passes over a resident strip.

    pref_t: [P, T*64] sbuf tile (destination-node vectors, updated in place
            for routing passes). Returns (final pref tile, last h tile,
            last alpha tile)."""
    prod_pool, small_pool = sb
    last_alpha = None
    h_t = None
    for ps in range(n_pass):
        is_final = ps == n_pass - 1
        prod = prod_pool.tile([P, S * C], F32, tag=f"prod{tag}")
        nc.vector.tensor_tensor(
            out=prod[:].rearrange("p (t d c) -> p t d c", t=T, d=D, c=C),
            in0=xj[:].rearrange("p (t d c) -> p t d c", t=T, d=D, c=C),
            in1=pref_t[:].rearrange("p (t c) -> p t c", t=T, c=C)
                .unsqueeze(2).to_broadcast([P, T, D, C]),
            op=ALU.mult)
        l_t = small_pool.tile([P, S], F32, tag=f"l{tag}")
        nc.vector.tensor_reduce(
            out=l_t[:].rearrange("p (t d) -> p t d", t=T, d=D),
            in_=prod[:].rearrange("p (t d c) -> p t d c", t=T, d=D, c=C),
            axis=AXX, op=ALU.add)
        e_t = small_pool.tile([P, S], F32, tag=f"e{tag}")
        nc.scalar.activation(out=e_t[:], in_=l_t[:], func=ACTF.Exp)
        nc.vector.tensor_tensor(out=e_t[:], in0=e_t[:], in1=mask_t[:],
                                op=ALU.mult)
        den = small_pool.tile([P, T], F32, tag=f"den{tag}")
        nc.vector.tensor_reduce(
            out=den[:], in_=e_t[:].rearrange("p (t d) -> p t d", t=T, d=D),
            axis=AXX, op=ALU.add)
        nc.vector.tensor_scalar_add(den[:], den[:], 1e-16)
        rec = small_pool.tile([P, T], F32, tag=f"rec{tag}")
        nc.vector.reciprocal(rec[:], den[:])
        alpha = small_pool.tile([P, S], F32, tag=f"alpha{tag}")
        nc.vector.tensor_tensor(
            out=alpha[:].rearrange("p (t d) -> p t d", t=T, d=D),
            in0=e_t[:].rearrange("p (t d) -> p t d", t=T, d=D),
            in1=rec[:].unsqueeze(2).to_broadcast([P, T, D]),
            op=ALU.mult)
        # h = sum_d alpha * xj
        prod2 = prod_pool.tile([P, S * C], F32, tag=f"prod{tag}")
        nc.vector.tensor_tensor(
            out=prod2[:].rearrange("p (t c d) -> p t c d", t=T, c=C, d=D),
            in0=xj[:].rearrange("p (t d c) -> p t c d", t=T, d=D, c=C),
            in1=alpha[:].rearrange("p (t d) -> p t d", t=T, d=D)
                .unsqueeze(2).to_broadcast([P, T, C, D]),
            op=ALU.mult)
        h_t = small_pool.tile([P, T * C], F32, tag=f"h{tag}")
        nc.vector.tensor_reduce(
            out=h_t[:].rearrange("p (t c) -> p t c", t=T, c=C),
            in_=prod2[:].rearrange("p (t c d) -> p t c d", t=T, c=C, d=D),
            axis=AXX, op=ALU.add)
        if not is_final:
            # pref = l2norm(pref + h)
            pnew = small_pool.tile([P, T * C], F32, tag=f"pref{tag}")
            nc.vector.tensor_tensor(out=pnew[:], in0=pref_t[:], in1=h_t[:],
                                    op=ALU.add)
            pref_t = _emit_l2norm(nc, small_pool, pnew, T, tag)
        else:
            last_alpha = alpha
    return pref_t, h_t, last_alpha


def _emit_l2norm(nc, small_pool, v_t, T, tag):
    """row-wise l2norm of [P, T*64] per (p, t) vector; returns new tile."""
    sq = small_pool.tile([P, T * C], F32, tag=f"sq{tag}")
    nc.vector.tensor_tensor(out=sq[:], in0=v_t[:], in1=v_t[:], op=ALU.mult)
    ss = small_pool.tile([P, T], F32, tag=f"ss{tag}")
    nc.vector.tensor_reduce(
        out=ss[:], in_=sq[:].rearrange("p (t c) -> p t c", t=T, c=C),
        axis=AXX, op=ALU.add)
    nrm = small_pool.tile([P, T], F32, tag=f"nrm{tag}")
    nc.scalar.activation(out=nrm[:], in_=ss[:], func=ACTF.Sqrt)
    nc.vector.tensor_scalar_max(nrm[:], nrm[:], 1e-12)
    rn = small_pool.tile([P, T], F32, tag=f"rn{tag}")
    nc.vector.reciprocal(rn[:], nrm[:])
    out = small_pool.tile([P, T * C], F32, tag=f"pref{tag}")
    nc.vector.tensor_tensor(
        out=out[:].rearrange("p (t c) -> p t c", t=T, c=C),
        in0=v_t[:].rearrange("p (t c) -> p t c", t=T, c=C),
        in1=rn[:].unsqueeze(2).to_broadcast([P, T, C]),
        op=ALU.mult)
    return out


def _emit_lrelu(nc, pool, v_t, shape, tag):
    out = pool.tile(shape, F32, tag=tag)
    nc.vector.scalar_tensor_tensor(out=out[:], in0=v_t[:], scalar=NEG_SLOPE,
                                   in1=v_t[:], op0=ALU.mult, op1=ALU.max)
    return out


def _emit_two_level_gather(nc, sb_pool, dram_pool, src_table, l1_sb, l2_sb,
                           l1_counts, S, tag):
    """two-level gather: windowed sorted-unique gathers into a compact DRAM
    table, then one slot-ordered gather. Returns xj tile [P, S*64]."""
    K_tot = sum(l1_counts) // P
    tab = sb_pool.tile([P, K_tot * C], F32, tag=f"tab{tag}")
    koff = 0
    icol = 0
    for wi, cnt in enumerate(l1_counts):
        if cnt == 0:
            continue
        kw = cnt // P
        nc.gpsimd.dma_gather(
            out_ap=tab[:, koff * C:(koff + kw) * C]
                .rearrange("p (k c) -> p k c", k=kw, c=C),
            in_ap=src_table[wi * WINDOW:, :],
            idxs_ap=l1_sb[:, icol:icol + cnt // 16],
            num_idxs=cnt, num_idxs_reg=cnt, elem_size=C,
            single_packet=False)
        koff += kw
        icol += cnt // 16
    tab_d = dram_pool.tile([P * K_tot, C], F32, tag=f"tabd{tag}")
    nc.sync.dma_start(
        out=tab_d[:].rearrange("(p k) c -> p (k c)", p=P, k=K_tot),
        in_=tab[:])
    xj = sb_pool.tile([P, S * C], F32, tag=f"xj{tag}")
    nc.gpsimd.dma_gather(
        out_ap=xj[:].rearrange("p (s c) -> p s c", s=S, c=C),
        in_ap=tab_d[:],
        idxs_ap=l2_sb[:],
        num_idxs=P * S, num_idxs_reg=P * S, elem_size=C,
        single_packet=False)
    return xj


def build_program(NI, NU, FDIM, fwd, rev):
    """Build the full 8-core SPMD program. Returns (nc, meta)."""
    CUT = int(os.environ.get("BASS_KERNEL_CUT", "9"))
    ipc = NI // NCORES
    ipc_pad = ((ipc + 511) // 512) * 512
    n_nt = ipc_pad // 512
    kch = FDIM // P                      # 768/128 = 6
    T_tot = fwd["T_tot"]
    TR_tot = rev["T_tot"]

    fwd_cols = int(sum(s * 1 for s in fwd["slot_cols"]))
    rev_cols = int(sum(rev["slot_cols"]))
    fwd_l1c = fwd["l1_idx"][0].shape[1]
    rev_l1c = rev["l1_idx"][0].shape[1]
    fwd_l2c = fwd["l2_idx"][0].shape[1]
    rev_l2c = rev["l2_idx"][0].shape[1]

    nc = bacc.Bacc("TRN2", target_bir_lowering=False, debug=False,
                   num_devices=NCORES)

    featT = nc.dram_tensor("featT", [FDIM, ipc_pad], F32, kind="ExternalInput")
    Wt = nc.dram_tensor("Wt", [FDIM, P], F32, kind="ExternalInput")
    bvec = nc.dram_tensor("bvec", [C, 1], F32, kind="ExternalInput")
    pref_in = nc.dram_tensor("pref_in", [P, T_tot * C], F32, kind="ExternalInput")
    fwd_l1 = nc.dram_tensor("fwd_l1", [P, fwd_l1c], I16, kind="ExternalInput")
    fwd_l2 = nc.dram_tensor("fwd_l2", [P, fwd_l2c], I16, kind="ExternalInput")
    fwd_mask = nc.dram_tensor("fwd_mask", [P, fwd_cols], F32, kind="ExternalInput")
    rev_l1 = nc.dram_tensor("rev_l1", [P, rev_l1c], I16, kind="ExternalInput")
    rev_l2 = nc.dram_tensor("rev_l2", [P, rev_l2c], I16, kind="ExternalInput")
    rev_mask = nc.dram_tensor("rev_mask", [P, rev_cols], F32, kind="ExternalInput")
    rev_xi = nc.dram_tensor("rev_xi", [P, 8 * TR_tot], I16, kind="ExternalInput")

    out_user = nc.dram_tensor("out_user", [P, T_tot * C], F32, kind="ExternalOutput")
    out_item = nc.dram_tensor("out_item", [P, TR_tot * C], F32, kind="ExternalOutput")
    alpha_fwd = nc.dram_tensor("alpha_fwd", [P, fwd_cols], F32, kind="ExternalOutput")
    alpha_rev = nc.dram_tensor("alpha_rev", [P, rev_cols], F32, kind="ExternalOutput")

    feats_local = nc.dram_tensor("feats_local", [ipc_pad, C], F32)
    feats_full = nc.dram_tensor("feats_full", [NCORES * ipc_pad, C], F32,
                                addr_space="Shared")
    pref_local = nc.dram_tensor("pref_local", [P * T_tot, C], F32)
    pref_full = nc.dram_tensor("pref_full", [NCORES * P * T_tot, C], F32,
                               addr_space="Shared")

    with tile.TileContext(nc) as tc:
        with (
            tc.tile_pool(name="mm", bufs=3) as mm_pool,
            tc.tile_pool(name="ps", bufs=2, space="PSUM") as ps_pool,
            tc.tile_pool(name="big", bufs=2) as big_pool,
            tc.tile_pool(name="small", bufs=3) as small_pool,
            tc.tile_pool(name="dram", bufs=2, space="DRAM") as dram_pool,
        ):
            # ---------------- phase 0: feats = l2norm(lrelu(feature@W.T+b))
            ident = small_pool.tile([P, P], F32, tag="ident")
            make_identity(nc, ident[:])
            bias_dma = small_pool.tile([C, 1], F32, tag="biasd")
            nc.sync.dma_start(bias_dma[:], bvec[:])
            # DVE-produced copy: keeps the per-tile bias-add at <=1 sync wait
            bias_t = small_pool.tile([C, 1], F32, tag="bias")
            nc.vector.tensor_copy(bias_t[:], bias_dma[:])
            wt_t = mm_pool.tile([P, kch * P], F32, tag="wt")
            nc.sync.dma_start(
                wt_t[:].rearrange("p (k c) -> p k c", k=kch, c=P),
                Wt[:].rearrange("(k p) c -> k p c", k=kch, p=P)
                .transpose([1, 0, 2]))
            # PE allows only ONE sync wait per matmul: chain two dummy
            # transposes so ident and the weight DMA are observed one at a
            # time before the real matmul loop (which then only waits on ft).
            warm = ps_pool.tile([P, P], F32, tag="warm")
            nc.tensor.transpose(warm[:], ident[:], ident[:])
            warm2 = ps_pool.tile([P, P], F32, tag="warm")
            nc.tensor.transpose(warm2[:], wt_t[:, :P], ident[:])
            for nt in range(n_nt):
                psumT = ps_pool.tile([P, 512], F32, tag="psT")
                for k in range(kch):
                    ft = mm_pool.tile([P, 512], F32, tag="ft")
                    nc.sync.dma_start(
                        ft[:], featT[k * P:(k + 1) * P, nt * 512:(nt + 1) * 512])
                    nc.tensor.matmul(
                        psumT[:],
                        lhsT=wt_t[:, k * P:(k + 1) * P],
                        rhs=ft[:],
                        start=(k == 0), stop=(k == kch - 1))
                fT = mm_pool.tile([C, 512], F32, tag="fT")
                # lrelu(x + b)  (ACT does copy+bias; avoids TensorScalarPtr)
                nc.scalar.activation(out=fT[:], in_=psumT[:C, :],
                                     func=ACTF.Identity, bias=bias_t[:, 0:1])
                nc.vector.scalar_tensor_tensor(
                    out=fT[:], in0=fT[:], scalar=NEG_SLOPE, in1=fT[:],
                    op0=ALU.mult, op1=ALU.max)
                for q in range(4):
                    pst = ps_pool.tile([P, C], F32, tag="pst")
                    nc.tensor.transpose(pst[:], fT[:, q * P:(q + 1) * P],
                                        ident[:C, :C])
                    sq = mm_pool.tile([P, C], F32, tag="sqf")
                    nc.scalar.activation(out=sq[:], in_=pst[:],
                                         func=ACTF.Square)
                    ssf = mm_pool.tile([P, 1], F32, tag="ssf")
                    nc.vector.tensor_reduce(out=ssf[:], in_=sq[:], axis=AXX,
                                            op=ALU.add)
                    nc.scalar.activation(out=ssf[:], in_=ssf[:], func=ACTF.Sqrt)
                    nc.vector.tensor_scalar_max(ssf[:], ssf[:], 1e-12)
                    rnf = mm_pool.tile([P, 1], F32, tag="rnf")
                    nc.vector.reciprocal(rnf[:], ssf[:])
                    fout = mm_pool.tile([P, C], F32, tag="fout")
                    nc.vector.tensor_tensor(
                        out=fout[:], in0=pst[:],
                        in1=rnf[:, 0:1].to_broadcast([P, C]), op=ALU.mult)
                    nc.sync.dma_start(
                        feats_local[nt * 512 + q * P: nt * 512 + (q + 1) * P, :],
                        fout[:])

            nc.gpsimd.collective_compute(
                "AllGather", ALU.bypass, replica_groups=[list(range(NCORES))],
                ins=[feats_local[:]], outs=[feats_full[:]])
            tc.strict_bb_all_engine_barrier()

            if CUT < 2:
                nc.compile()
                return nc
            # ---------------- phase 1: forward side (3 routing + final)
            m_off = 0
            l1_off = 0
            l2_off = 0
            t_off = 0
            for si, (t0, T, D) in enumerate(fwd["strips"]):
                S = T * D
                l1c = sum(fwd["l1_counts"][si]) // 16
                l1_sb = small_pool.tile([P, max(l1c, 16)], I16, tag="l1sb")
                if l1c:
                    nc.sync.dma_start(l1_sb[:, :l1c],
                                      fwd_l1[:, l1_off:l1_off + l1c])
                l2_sb = small_pool.tile([P, 8 * S], I16, tag="l2sb")
                nc.sync.dma_start(l2_sb[:], fwd_l2[:, l2_off:l2_off + 8 * S])
                mask_t = big_pool.tile([P, S], F32, tag="maskf")
                nc.sync.dma_start(mask_t[:], fwd_mask[:, m_off:m_off + S])
                pref_t = small_pool.tile([P, T * C], F32, tag="pref")
                nc.sync.dma_start(pref_t[:],
                                  pref_in[:, t_off * C:(t_off + T) * C])
                if CUT >= 3:
                    xj = _emit_two_level_gather(
                        nc, big_pool, dram_pool, feats_full, l1_sb, l2_sb,
                        fwd["l1_counts"][si], S, "f")
                else:
                    xj = big_pool.tile([P, S * C], F32, tag="xjf")
                    nc.vector.memset(xj[:], 0.125)

                # initial l2norm of preference
                pref_t = _emit_l2norm(nc, small_pool, pref_t, T, "")
                pref_t, h_t, alpha = _emit_strip_passes(
                    nc, (big_pool, small_pool), pref_t, xj, mask_t, T, D, S,
                    n_pass=4, tag="")
                # outputs
                nc.sync.dma_start(alpha_fwd[:, m_off:m_off + S], alpha[:])
                h_act = _emit_lrelu(nc, small_pool, h_t, [P, T * C], "hact")
                ou = small_pool.tile([P, T * C], F32, tag="ou")
                nc.vector.tensor_tensor(out=ou[:], in0=pref_t[:], in1=h_act[:],
                                        op=ALU.add)
                nc.sync.dma_start(out_user[:, t_off * C:(t_off + T) * C], ou[:])
                # pref_local rows p*T_tot + (t_off + t)
                nc.sync.dma_start(
                    pref_local[:]
                    .rearrange("(p t) c -> p t c", p=P, t=T_tot)[:, t_off:t_off + T, :],
                    pref_t[:].rearrange("p (t c) -> p t c", t=T, c=C))
                m_off += S
                l1_off += l1c
                l2_off += 8 * S
                t_off += T

            if CUT < 4:
                nc.compile()
                return nc
            nc.gpsimd.collective_compute(
                "AllGather", ALU.bypass, replica_groups=[list(range(NCORES))],
                ins=[pref_local[:]], outs=[pref_full[:]])
            tc.strict_bb_all_engine_barrier()

            if CUT < 5:
                nc.compile()
                return nc
            # ---------------- phase 2: reverse side (items aggregate users)
            m_off = 0
            l1_off = 0
            l2_off = 0
            t_off = 0
            for si, (t0, T, D) in enumerate(rev["strips"]):
                S = T * D
                l1c = sum(rev["l1_counts"][si]) // 16
                l1_sb = small_pool.tile([P, max(l1c, 16)], I16, tag="l1sb")
                if l1c:
                    nc.sync.dma_start(l1_sb[:, :l1c],
                                      rev_l1[:, l1_off:l1_off + l1c])
                l2_sb = small_pool.tile([P, 8 * S], I16, tag="l2sb")
                nc.sync.dma_start(l2_sb[:], rev_l2[:, l2_off:l2_off + 8 * S])
                mask_t = big_pool.tile([P, S], F32, tag="maskf")
                nc.sync.dma_start(mask_t[:], rev_mask[:, m_off:m_off + S])
                xi_sb = small_pool.tile([P, 8 * T], I16, tag="xisb")
                nc.sync.dma_start(xi_sb[:], rev_xi[:, 8 * t_off:8 * (t_off + T)])
                xi_t = small_pool.tile([P, T * C], F32, tag="pref")
                nc.gpsimd.dma_gather(
                    out_ap=xi_t[:].rearrange("p (t c) -> p t c", t=T, c=C),
                    in_ap=feats_local[:], idxs_ap=xi_sb[:],
                    num_idxs=P * T, num_idxs_reg=P * T, elem_size=C,
                    single_packet=False)
                xj = _emit_two_level_gather(
                    nc, big_pool, dram_pool, pref_full, l1_sb, l2_sb,
                    rev["l1_counts"][si], S, "f")
                _, h_t, alpha = _emit_strip_passes(
                    nc, (big_pool, small_pool), xi_t, xj, mask_t, T, D, S,
                    n_pass=1, tag="")
                nc.sync.dma_start(alpha_rev[:, m_off:m_off + S], alpha[:])
                h_act = _emit_lrelu(nc, small_pool, h_t, [P, T * C], "hact")
                oi = small_pool.tile([P, T * C], F32, tag="ou")
                nc.vector.tensor_tensor(out=oi[:], in0=xi_t[:], in1=h_act[:],
                                        op=ALU.add)
                nc.sync.dma_start(out_item[:, t_off * C:(t_off + T) * C], oi[:])
                m_off += S
                l1_off += l1c
                l2_off += 8 * S
                t_off += T

    nc.compile()
    return nc


def _run_timed(nc, in_maps, reps):
    """Execute the program on 8 cores via PJRT, with inputs device-resident,
    timing `reps` re-executions (wall clock, min over reps)."""
    import time as _time

    import jax
    from jax.experimental.shard_map import shard_map
    from jax.sharding import Mesh, PartitionSpec

    from concourse import mybir as _mb
    from concourse.bass2jax import (_bass_exec_p, install_neuronx_cc_hook,
                                    partition_id_tensor)

    install_neuronx_cc_hook()
    n_cores = len(in_maps)
    partition_name = (nc.partition_id_tensor.name
                      if nc.partition_id_tensor else None)
    in_names, out_names, out_avals, zero_outs = [], [], [], []
    for alloc in nc.m.functions[0].allocations:
        if not isinstance(alloc, _mb.MemoryLocationSet):
            continue
        name = alloc.memorylocations[0].name
        if alloc.kind == "ExternalInput":
            if name != partition_name:
                in_names.append(name)
        elif alloc.kind == "ExternalOutput":
            shape = tuple(alloc.tensor_shape)
            dtype = _mb.dt.np(alloc.dtype)
            out_names.append(name)
            out_avals.append(jax.core.ShapedArray(shape, dtype))
            zero_outs.append(np.zeros(shape, dtype))
    n_params = len(in_names)
    in_names.extend(out_names)
    if partition_name is not None:
        in_names.append(partition_name)

    def _body(*args):
        operands = list(args)
        if partition_name is not None:
            operands.append(partition_id_tensor())
        outs = _bass_exec_p.bind(
            *operands, out_avals=tuple(out_avals), in_names=tuple(in_names),
            out_names=tuple(out_names), lowering_input_output_aliases=(),
            sim_require_finite=True, sim_require_nnan=True, nc=nc)
        return tuple(outs)

    devices = jax.devices()[:n_cores]
    mesh = Mesh(np.asarray(devices), ("core",))
    n_outs = len(out_names)
    sharded = jax.jit(
        shard_map(_body, mesh=mesh,
                  in_specs=(PartitionSpec("core"),) * (n_params + n_outs),
                  out_specs=(PartitionSpec("core"),) * n_outs,
                  check_rep=False),
        keep_unused=True)
    concat_in = [
        np.concatenate([np.asarray(in_maps[c][nm]) for c in range(n_cores)],
                       axis=0) for nm in in_names[:n_params]]
    concat_zeros = [
        np.zeros((n_cores * z.shape[0], *z.shape[1:]), z.dtype)
        for z in zero_outs]
    sharding = jax.sharding.NamedSharding(mesh, PartitionSpec("core"))
    dev_in = [jax.device_put(a, sharding) for a in concat_in]
    dev_zeros = [jax.device_put(a, sharding) for a in concat_zeros]

    out_arrs = sharded(*dev_in, *dev_zeros)     # warm-up + result
    jax.block_until_ready(out_arrs)
    results = [
        {name: np.asarray(out_arrs[i]).reshape(n_cores, *out_avals[i].shape)[c]
         for i, name in enumerate(out_names)}
        for c in range(n_cores)]

    times = []
    for _ in range(reps):
        t0 = _time.perf_counter()
        r = sharded(*dev_in, *dev_zeros)
        jax.block_until_ready(r)
        times.append(_time.perf_counter() - t0)
    return results, times


# ----------------------------------------------------------------------------
# top level
# ----------------------------------------------------------------------------

def kernel(feature, preference, W, b, edge_index):
    feature = np.asarray(feature)
    preference = np.asarray(preference)
    W = np.asarray(W)
    b = np.asarray(b)
    edge_index = np.asarray(edge_index)

    NI, FDIM = feature.shape
    NU = preference.shape[0]
    E = edge_index.shape[1]
    src = edge_index[0].astype(np.int64)          # items (global: NU..NU+NI)
    dst = edge_index[1].astype(np.int64)          # users
    src_it = src - NU                              # item local 0..NI

    ipc = NI // NCORES
    ipc_pad = ((ipc + 511) // 512) * 512
    upc = NU // NCORES

    # table row of item i in feats_full
    def item_row(it):
        c = it // ipc
        return c * ipc_pad + (it - c * ipc)

    eids = np.arange(E, dtype=np.int64)
    fwd = _build_side(dst, item_row(src_it), eids, NU, NCORES * ipc_pad)

    # pref_full row of user u (needs fwd ords): c*P*T_tot + p*T_tot + t
    T_tot = fwd["T_tot"]

    def user_row(u):
        c = u // upc
        k = np.empty(len(u), np.int64)
        for cc in range(NCORES):
            m = c == cc
            k[m] = fwd["invs"][cc][u[m] - cc * upc]
        return c * P * T_tot + (k % P) * T_tot + (k // P)

    rev = _build_side(src_it, user_row(dst), eids, NI, NCORES * P * T_tot)

    nc = build_program(NI, NU, FDIM, fwd, rev)

    # ---------------- input maps
    featureT = np.ascontiguousarray(feature.T)     # [FDIM, NI]
    Wt_np = np.zeros((FDIM, P), np.float32)        # fp32r needs 128-col tiling
    Wt_np[:, :C] = W.T
    in_maps = []
    for c in range(NCORES):
        ftc = np.zeros((FDIM, ipc_pad), np.float32)
        ftc[:, :ipc] = featureT[:, c * ipc:(c + 1) * ipc]
        # pref layout [P, T_tot*C]: (p, t) = sorted pos t*P+p
        pp = np.zeros((P, T_tot, 64), np.float32)
        po = preference[c * upc:(c + 1) * upc][fwd["ords"][c]]  # sorted order
        po = np.concatenate([po, np.zeros((fwd["npc_pad"] - upc, 64), np.float32)])
        pp[:, :, :] = po.reshape(T_tot, P, 64).transpose(1, 0, 2)
        # rev xi idx: local item ids in tile grid, wrapped per strip
        xi_parts = []
        for (t0, T, D) in rev["strips"]:
            k = np.arange(t0 * P, (t0 + T) * P)
            ids = rev["ords"][c][np.minimum(k, rev["npc"] - 1)]
            ids = np.where(k < rev["npc"], ids, 0)
            # gather pos i = t*P + p  -> wrapped
            xi_parts.append(_wrap_idx(ids, T * P))
        in_maps.append({
            "featT": ftc,
            "Wt": Wt_np.astype(np.float32),
            "bvec": b.reshape(C, 1).astype(np.float32),
            "pref_in": pp.reshape(P, T_tot * 64),
            "fwd_l1": fwd["l1_idx"][c], "fwd_l2": fwd["l2_idx"][c],
            "fwd_mask": fwd["masks"][c],
            "rev_l1": rev["l1_idx"][c], "rev_l2": rev["l2_idx"][c],
            "rev_mask": rev["masks"][c],
            "rev_xi": np.concatenate(xi_parts, axis=1),
        })

    # ---------------- run
    reps = int(os.environ.get("BASS_KERNEL_TIMEIT", "0"))
    if reps > 0:
        results, times = _run_timed(nc, in_maps, reps)
        _last_run_info["exec_time_ns"] = int(min(times) * 1e9)
        _last_run_info["times"] = times
    elif os.environ.get("BASS_KERNEL_SIM", "0") == "1":
        from concourse.bass_interp import MultiCoreSim
        sim = MultiCoreSim(nc, NCORES)
        for c in range(NCORES):
            for k, v in in_maps[c].items():
                sim.cores[c].tensor(k)[:] = v
        sim.simulate(check_with_hw=False)
        results = [
            {k: np.array(sim.cores[c].tensor(k))
             for k in ("out_user", "out_item", "alpha_fwd", "alpha_rev")}
            for c in range(NCORES)]
        _last_run_info["exec_time_ns"] = None
    else:
        from concourse.bass_utils import run_bass_kernel_spmd
        trace = os.environ.get("BASS_KERNEL_TRACE", "0") == "1"
        res = run_bass_kernel_spmd(nc, in_maps, list(range(NCORES)),
                                   trace=trace)
        _last_run_info["exec_time_ns"] = res.exec_time_ns
        _last_run_info["profile_json"] = res.profile_json
        results = res.results

    # ---------------- assemble outputs
    out0 = np.zeros((NU + NI, C), np.float32)
    alpha = np.zeros((2 * E, 1), np.float32)
    for c in range(NCORES):
        r = results[c]
        ou = r["out_user"].reshape(P, T_tot, C)
        k = np.arange(upc)
        p = fwd["invs"][c][k] % P
        t = fwd["invs"][c][k] // P
        out0[c * upc + k] = ou[p, t]
        oi = r["out_item"].reshape(P, rev["T_tot"], C)
        ki = np.arange(ipc)
        pi = rev["invs"][c][ki] % P
        ti = rev["invs"][c][ki] // P
        out0[NU + c * ipc + ki] = oi[pi, ti]
        fe = fwd["slot_eids"][c].ravel()
        fv = fe >= 0
        alpha[fe[fv], 0] = r["alpha_fwd"].ravel()[fv]
        re_ = rev["slot_eids"][c].ravel()
        rv = re_ >= 0
        alpha[E + re_[rv], 0] = r["alpha_rev"].ravel()[rv]

    return out0, alpha
